# revision 2
# baseline (speedup 1.0000x reference)
"""Trainium2 Bass kernel for nn_Attentive: out = x * w (per-channel scale).

x: (8192, 4096) f32, w: (4096,) f32. Data-parallel over 8 NeuronCores:
each core handles 1024 contiguous rows of x; w is replicated to every core.

Per core the kernel streams 16 MiB in + 16 MiB out of HBM (memory-bound).
Measured ~86 us/core on hardware (~390 GB/s effective mixed read+write
streaming; pure-load and pure-store rates measured at 461/565 GB/s, so the
binding constraint is the mixed-direction SDMA/HBM rate).

Design: the core's 1024 rows form 8 chunks of [128 partitions, 4096]. Loads
are grouped 2 chunks per DMA (4 MiB, descriptor overheads amortized), the
per-channel scale runs on the vector engine (in-place, chunk at a time,
fully hidden under DMA), stores mirror the loads on the second HWDGE ring.
bufs=4 slots give the Tile scheduler a deep load/compute/store pipeline.
"""

import os
import time

import numpy as np

# No NTFF hook is available under this container's axon stub; a trace request
# would crash inside run_bass_kernel_spmd. Force-disable tracing defensively.
os.environ.setdefault("BASS_NEVER_TRACE", "1")

import concourse.bacc as bacc
import concourse.mybir as mybir
from concourse.bass_utils import run_bass_kernel_spmd
from concourse.tile import TileContext

NTOK = 8192
ISIZE = 4096
NCORES = 8
ROWS = NTOK // NCORES  # 1024 rows of x per core
P = 128  # SBUF partitions
NCHUNKS = ROWS // P  # 8 chunks of [128, 4096] (2 MiB each) per core

_nc_cache = None


def _build_nc(repeat: int = 1, loop: int = 1, bufs: int = 4,
              plan=(2, 2, 2, 2)):
    """Build the per-core Bass program.

    `plan` partitions the 8 row-chunks into load tiles (e.g. (2,2,2,2) =
    four 4 MiB loads). `repeat` python-unrolls the body and `loop` wraps it
    in a hardware For_i loop; both re-run the same idempotent computation
    and exist only for wall-clock benchmarking (the graded path uses 1/1).
    """
    assert sum(plan) == NCHUNKS
    nc = bacc.Bacc("TRN2", target_bir_lowering=False, num_devices=NCORES)
    x = nc.dram_tensor("x", [ROWS, ISIZE], mybir.dt.float32, kind="ExternalInput")
    w = nc.dram_tensor("w", [ISIZE], mybir.dt.float32, kind="ExternalInput")
    out = nc.dram_tensor("out", [ROWS, ISIZE], mybir.dt.float32,
                         kind="ExternalOutput")

    # chunk n = rows [n*128, (n+1)*128)
    xv = x.rearrange("(n p) m -> n p m", p=P)
    ov = out.rearrange("(n p) m -> n p m", p=P)

    with TileContext(nc) as tc:
        with (
            tc.tile_pool(name="wpool", bufs=1) as wpool,
            tc.tile_pool(name="sbuf", bufs=bufs) as pool,
        ):
            # Replicate w across all 128 partitions once (~2 MiB, one-time).
            # On the ACT HWDGE ring: the store ring is idle at kernel start so
            # this never queues ahead of the first x load, and avoiding gpsimd
            # keeps POOL out of the NEFF's engine set (smaller preamble/drain).
            w_sb = wpool.tile([P, ISIZE], mybir.dt.float32)
            nc.scalar.dma_start(out=w_sb[:], in_=w[None, :].to_broadcast((P, ISIZE)))

            def body():
                for _ in range(repeat):
                    base = 0
                    for cpt in plan:
                        t = pool.tile([P, max(plan), ISIZE], mybir.dt.float32,
                                      tag="t")
                        # one DMA for chunks [base, base+cpt): DRAM dims
                        # p(row-in-chunk), n(chunk), m -> SBUF [P, cpt, ISIZE]
                        src = xv[base : base + cpt].rearrange("n p m -> p n m")
                        nc.sync.dma_start(out=t[:, :cpt], in_=src)
                        for c in range(cpt):
                            nc.vector.tensor_mul(t[:, c], t[:, c], w_sb[:])
                        dst = ov[base : base + cpt].rearrange("n p m -> p n m")
                        nc.scalar.dma_start(out=dst, in_=t[:, :cpt])
                        base += cpt

            if loop > 1:
                with tc.For_i(0, loop, 1):
                    body()
            else:
                body()
    nc.compile()
    return nc


def _make_in_maps(x: np.ndarray, w: np.ndarray):
    x = np.ascontiguousarray(np.asarray(x, dtype=np.float32))
    w = np.ascontiguousarray(np.asarray(w, dtype=np.float32))
    return [
        {"x": x[c * ROWS : (c + 1) * ROWS], "w": w} for c in range(NCORES)
    ]


def kernel(x: np.ndarray, w: np.ndarray) -> np.ndarray:
    global _nc_cache
    x = np.ascontiguousarray(np.asarray(x, dtype=np.float32))
    w = np.ascontiguousarray(np.asarray(w, dtype=np.float32))
    assert x.shape == (NTOK, ISIZE) and w.shape == (ISIZE,)

    if _nc_cache is None:
        _nc_cache = _build_nc()
    nc = _nc_cache

    in_maps = _make_in_maps(x, w)
    # The axon-tunneled terminals occasionally die mid-run
    # (NRT_EXEC_UNIT_UNRECOVERABLE) and the pool takes ~1 min to swap in a
    # fresh one. Retry with backoff rather than failing the whole call.
    last_exc = None
    for attempt in range(3):
        if attempt:
            time.sleep(45)
            try:
                import jax

                jax.clear_caches()
                clear_backends = getattr(jax, "clear_backends", None)
                if clear_backends is not None:
                    clear_backends()
            except Exception:
                pass
        try:
            res = run_bass_kernel_spmd(nc, in_maps, core_ids=list(range(NCORES)))
            return np.concatenate([r["out"] for r in res.results], axis=0)
        except Exception as exc:  # noqa: BLE001 - device loss is not typed
            last_exc = exc
    raise last_exc

